# revision 13
# baseline (speedup 1.0000x reference)
"""DeltaNet Trainium2 kernel (nn_DeltaNet_41961830482331), v3.

Full module: qkv = x @ w_attn; per-(head,dim-group) standardization (ddof=1);
DeltaNet recurrence  S_t = S_{t-1}(0.99 I - 0.01 k k^T) + k v^T, o_t = S_t q_t;
y = o @ w_proj; out = x + y.

Sharding: 8 cores = 4 batches x 2 head-groups (6 heads each); host sums the two
partial y-projections per batch plus the residual x.

Chunked recurrence (n=128, b' = beta/gamma, st = Sh0^T, 9-term Neumann solve):
  G = K K^T; Gu = striu(G), Gl = stril(G); N = b' Gl
  Rraw = VKm^T K + K st (VKm = striu(Vh K^T)); R = b' Rraw
  n2 = b'^2 Gl^T Gu (=N2^T); n3 = -b' Gl^T n2 (=-N3^T)
  z0 = R - N R + N2 R; w1 = z0 + n3^T z0; M = z0 + n3^T w1
  O^T = K^T VQm - M^T KQm + st^T Qh^T;  st' = g^n (st + Vh^T K - K^T M)

Emission is software-pipelined: B-stages (state-dependent) of chunk c are
interleaved with A-stages of chunk c+1 so the PE queue never sits behind the
stats/normalize/mask chains. All matmul operands bf16; exact f32 scales are
folded into PSUM-evacuation ops; masks are 0/1 bf16 constants applied during
evacuation (vector/gpsimd split by head parity).
"""

import numpy as np

B, T, C = 4, 1024, 768
NH, HS = 12, 64
HPC = NH // 2            # heads per core
GAMMA, BETA = 0.99, 0.01
BP = BETA / GAMMA        # beta'
NC_ = 128                # chunk length n
NCH = T // NC_           # chunks
GN = GAMMA ** NC_        # gamma^n
NIT = 2                  # N^3 applications -> 3 + 3*NIT Neumann terms

_cache: dict = {}


def _build_program():
    import concourse.bass as bass
    import concourse.tile as tile
    from concourse import bacc, mybir

    f32 = mybir.dt.float32
    bf16 = mybir.dt.bfloat16
    Alu = mybir.AluOpType
    Act = mybir.ActivationFunctionType

    nc = bacc.Bacc()

    # ---- DRAM (per-core; SPMD same names on all cores) ----
    xT = nc.dram_tensor("xT", [128, NCH * 768], bf16, kind="ExternalInput")
    wA = nc.dram_tensor("wA", [128, 6 * 1152], bf16, kind="ExternalInput")
    wP = nc.dram_tensor("wP", [128, 3 * 768], bf16, kind="ExternalInput")
    gvec = nc.dram_tensor("gvec", [128, 2], f32, kind="ExternalInput")
    cid = nc.dram_tensor("cid", [128, 128], bf16, kind="ExternalInput")
    cmask = nc.dram_tensor("cmask", [128, 512], f32, kind="ExternalInput")
    y = nc.dram_tensor("y", [T, C], bf16, kind="ExternalOutput")

    xT3 = xT.rearrange("p (c j) -> p c j", c=NCH)
    wA3 = wA.rearrange("p (k j) -> p k j", k=6)

    with tile.TileContext(nc) as tc:
        with (
            tc.tile_pool(name="persist", bufs=1) as persist,
            tc.tile_pool(name="statp", bufs=3) as statp,
            tc.tile_pool(name="qsp", bufs=3) as qsp,
            tc.tile_pool(name="natp", bufs=3) as natp,
            tc.tile_pool(name="tp", bufs=3) as tp,
            tc.tile_pool(name="gramp", bufs=3) as gramp,
            tc.tile_pool(name="solvep", bufs=3) as solvep,
            tc.tile_pool(name="stp", bufs=2) as stp,
            tc.tile_pool(name="outp", bufs=3) as outp,
            tc.tile_pool(name="yp", bufs=2) as yp,
            tc.tile_pool(name="ps_q", bufs=2, space="PSUM") as ps_q,
            tc.tile_pool(name="ps_g", bufs=3, space="PSUM") as ps_g,
            tc.tile_pool(name="ps_nn", bufs=3, space="PSUM") as ps_nn,
        ):
            # ---- persistent loads (split across SP and Pool DMA rings) ----
            wA_sb = persist.tile([128, 6, 1152], bf16)
            xc_sb = []
            t_ = persist.tile([128, 768], bf16, tag="xc0", name="xc0")
            nc.gpsimd.dma_start(out=t_, in_=xT3[:, 0, :])
            xc_sb.append(t_)
            for k in range(6):
                eng = nc.sync if k % 2 == 0 else nc.gpsimd
                eng.dma_start(out=wA_sb[:, k, :], in_=wA3[:, k, :])
            id_sb = persist.tile([128, 128], bf16)
            nc.gpsimd.dma_start(out=id_sb, in_=cid[:, :])
            msk = persist.tile([128, 512], f32)
            nc.sync.dma_start(out=msk, in_=cmask[:, :])
            gv = persist.tile([128, 2], f32)
            nc.gpsimd.dma_start(out=gv, in_=gvec[:, :])
            for c in range(1, NCH):
                t_ = persist.tile([128, 768], bf16, tag=f"xc{c}", name=f"xc{c}")
                (nc.sync if c % 2 == 0 else nc.gpsimd).dma_start(out=t_, in_=xT3[:, c, :])
                xc_sb.append(t_)
            wP_sb = persist.tile([128, 3, 768], bf16)
            nc.gpsimd.dma_start(out=wP_sb, in_=wP.rearrange("p (k j) -> p k j", k=3))

            st_prev = stp.tile([128, 192], bf16, tag="st0")
            nc.vector.memset(st_prev, 0.0)

            ctx = [dict() for _ in range(NCH)]
            st_list = [st_prev]

            def stage_qkv_mm(c):
                """qkv matmuls + quick evac + bn_stats (2 chunks ahead)."""
                X = ctx[c]
                qs = []
                bn = statp.tile([128, 18, 8], f32, tag="bn")
                for nblk in range(3):
                    pq = ps_q.tile([128, 384], f32, tag="q", name=f"pq{nblk}")
                    for k in range(6):
                        nc.tensor.matmul(
                            pq[:, :],
                            lhsT=xc_sb[c][:, 128 * k:128 * k + 128],
                            rhs=wA_sb[:, k, 384 * nblk:384 * nblk + 384],
                            start=(k == 0), stop=(k == 5),
                        )
                    q_ = qsp.tile([128, 384], bf16, tag=f"qs{nblk}", name=f"qs{nblk}")
                    nc.scalar.copy(q_, pq)
                    for g in range(6):
                        nc.vector.bn_stats(bn[:, 6 * nblk + g, 0:6],
                                           q_[:, 64 * g:64 * g + 64])
                    qs.append(q_)
                X["qs"], X["bn"] = qs, bn

            def stage_qkv_norm(c):
                """grouped stats postprocessing + broadcast normalize."""
                X = ctx[c]
                qs, bn = X["qs"], X["bn"]

                def col(j):
                    return bn[:, :, j:j + 1].rearrange("p g o -> p (g o)")

                cvs = statp.tile([128, 18], f32, tag="cvs")
                nc.vector.tensor_add(cvs, col(2), col(5))
                dmn = statp.tile([128, 18], f32, tag="dmn")
                nc.vector.tensor_sub(dmn, col(1), col(4))
                dsq = statp.tile([128, 18], f32, tag="dsq")
                nc.vector.tensor_mul(dsq, dmn, dmn)
                var = statp.tile([128, 18], f32, tag="var")
                nc.vector.scalar_tensor_tensor(
                    out=var, in0=dsq, scalar=16.0, in1=cvs,
                    op0=Alu.mult, op1=Alu.add)
                # mu = 2*mean; rstd = 0.5/sqrt(var_unb) so (2x - mu)*rstd is exact
                sd = statp.tile([128, 18], f32, tag="sd")
                nc.scalar.activation(sd, var, Act.Sqrt, scale=4.0 / 63.0)
                rstd = statp.tile([128, 18], f32, tag="rstd")
                nc.vector.reciprocal(rstd, sd)
                mu = statp.tile([128, 18], f32, tag="mu")
                nc.vector.tensor_add(mu, col(1), col(4))
                nc.vector.tensor_scalar(
                    out=rstd[:, 0:6], in0=rstd[:, 0:6], scalar1=gv[:, 0:1],
                    scalar2=None, op0=Alu.mult)
                nc.vector.tensor_scalar(
                    out=rstd[:, 12:18], in0=rstd[:, 12:18], scalar1=gv[:, 1:2],
                    scalar2=None, op0=Alu.mult)

                knp = [natp.tile([128, 128], bf16, tag=f"knp{p}", name=f"knp{p}") for p in range(3)]
                vnp = [natp.tile([128, 128], bf16, tag=f"vnp{p}", name=f"vnp{p}") for p in range(3)]
                qnp = [natp.tile([128, 128], bf16, tag=f"qnp{p}", name=f"qnp{p}") for p in range(3)]
                dst = [qnp, knp, vnp]
                for nblk in range(3):
                    mu_b = mu[:, 6 * nblk:6 * nblk + 6].rearrange(
                        "p g -> p g ()").to_broadcast((128, 6, 64))
                    r_b = rstd[:, 6 * nblk:6 * nblk + 6].rearrange(
                        "p g -> p g ()").to_broadcast((128, 6, 64))
                    tmp = statp.tile([128, 384], bf16, tag=f"tmp{nblk}")
                    nc.vector.scalar_tensor_tensor(
                        out=tmp.rearrange("p (g d) -> p g d", d=64),
                        in0=qs[nblk].rearrange("p (g d) -> p g d", d=64),
                        scalar=2.0, in1=mu_b, op0=Alu.mult, op1=Alu.subtract)
                    for p in range(3):
                        nc.vector.tensor_mul(
                            dst[nblk][p].rearrange("p (g d) -> p g d", d=64),
                            tmp[:, 128 * p:128 * p + 128].rearrange(
                                "p (g d) -> p g d", d=64),
                            r_b[:, 2 * p:2 * p + 2, :])
                X["knp"], X["vnp"], X["qnp"] = knp, vnp, qnp

            def stage_tr(c):
                """transposes -> tsb = [kt | qt | vt] per pair."""
                X = ctx[c]
                tsb = []
                for p in range(3):
                    tps = ps_g.tile([128, 384], bf16, tag="g", name="tps")
                    nc.tensor.transpose(tps[:, 0:128], X["knp"][p], id_sb)
                    nc.tensor.transpose(tps[:, 128:256], X["qnp"][p], id_sb)
                    nc.tensor.transpose(tps[:, 256:384], X["vnp"][p], id_sb)
                    t_ = tp.tile([128, 384], bf16, tag=f"tsb{p}", name=f"tsb{p}")
                    nc.scalar.copy(t_, tps)
                    tsb.append(t_)
                X["tsb"] = tsb

            def stage_gram(c):
                """grams + masked evac -> gsb = [Gu | VKm | KQm | VQm | Gl]."""
                X = ctx[c]
                gsb = []
                for i in range(6):
                    p, sub = divmod(i, 2)
                    po = 64 * sub
                    ts = X["tsb"][p]
                    kt = ts[po:po + 64, 0:128]
                    vt = ts[po:po + 64, 256:384]
                    rhs2 = ts[po:po + 64, 0:256]
                    gps = ps_g.tile([128, 512], f32, tag="g", name="gps")
                    nc.tensor.matmul(gps[:, 0:256], lhsT=kt, rhs=rhs2)
                    nc.tensor.matmul(gps[:, 256:512], lhsT=vt, rhs=rhs2)
                    g_ = gramp.tile([128, 640], bf16, tag=f"gsb{i}", name=f"gsb{i}")
                    perm = gps.rearrange("p (i j f) -> p j i f", i=2, j=2)
                    nc.vector.tensor_mul(
                        g_[:, 0:512].rearrange("p (j i f) -> p j i f", j=2, i=2),
                        perm,
                        msk[:, 0:512].rearrange("p (j i f) -> p j i f", j=2, i=2))
                    nc.gpsimd.affine_select(
                        out=g_[:, 512:640], in_=g_[:, 0:128],
                        compare_op=Alu.is_ge, fill=0.0, base=-1,
                        pattern=[[-1, 128]], channel_multiplier=1)
                    nc.gpsimd.affine_select(
                        out=g_[:, 0:128], in_=g_[:, 0:128],
                        compare_op=Alu.is_ge, fill=0.0, base=-1,
                        pattern=[[1, 128]], channel_multiplier=-1)
                    gsb.append(g_)
                X["gsb"] = gsb

            def stage_nn(c):
                """n2 = b'^2 Gl^T Gu ; n3 = -b' Gl^T n2 (both bf16)."""
                X = ctx[c]
                n2, n3 = [], []
                for half in range(2):
                    pp = ps_nn.tile([128, 384], f32, tag="nn", name="n2p")
                    for j in range(3):
                        g_ = X["gsb"][3 * half + j]
                        nc.tensor.matmul(pp[:, 128 * j:128 * j + 128],
                                         lhsT=g_[:, 512:640], rhs=g_[:, 0:128])
                    t_ = solvep.tile([128, 384], bf16, tag=f"n2{half}", name=f"n2{half}")
                    nc.scalar.copy(t_, pp)
                    n2.append(t_)
                for half in range(2):
                    pp = ps_nn.tile([128, 384], f32, tag="nn", name="n3p")
                    for j in range(3):
                        g_ = X["gsb"][3 * half + j]
                        nc.tensor.matmul(pp[:, 128 * j:128 * j + 128],
                                         lhsT=g_[:, 512:640],
                                         rhs=n2[half][:, 128 * j:128 * j + 128])
                    t_ = solvep.tile([128, 384], bf16, tag=f"n3{half}", name=f"n3{half}")
                    nc.scalar.copy(t_, pp)
                    n3.append(t_)
                X["n2"], X["n3"] = n2, n3

            def stage_r(c):
                """R = b'(VKm^T K + K st); also rbp = -b'^2 Rraw."""
                X = ctx[c]
                rp = ps_nn.tile([128, 384], f32, tag="nn", name="rp")
                for i in range(6):
                    p, sub = divmod(i, 2)
                    po = 64 * sub
                    sl = slice(64 * i, 64 * i + 64)
                    nc.tensor.matmul(rp[:, sl], lhsT=X["gsb"][i][:, 128:256],
                                     rhs=X["knp"][p][:, po:po + 64],
                                     start=True, stop=False)
                    nc.tensor.matmul(rp[:, sl],
                                     lhsT=X["tsb"][p][po:po + 64, 0:128],
                                     rhs=st_list[c][po:po + 64, 64 * p:64 * p + 64],
                                     start=False, stop=True)
                r_ = solvep.tile([128, 384], bf16, tag="rsb", name="rsb")
                nc.scalar.mul(r_, rp, BP)
                X["r"] = r_

            def stage_z0(c):
                """z0 = R - N R + N2 R."""
                X = ctx[c]
                zp = ps_nn.tile([128, 384], f32, tag="nn", name="zp")
                for i in range(6):
                    sl = slice(64 * i, 64 * i + 64)
                    nc.tensor.matmul(zp[:, sl], lhsT=X["gsb"][i][:, 0:128],
                                     rhs=X["r"][:, sl], start=True, stop=False)
                    nc.tensor.matmul(zp[:, sl],
                                     lhsT=X["n2"][i // 3][:, 128 * (i % 3):128 * (i % 3) + 128],
                                     rhs=X["r"][:, sl], start=False, stop=True)
                z0 = solvep.tile([128, 384], bf16, tag="z0", name="z0")
                nc.vector.tensor_add(z0, zp, X["r"])
                X["z0"] = z0

            def stage_w(c, it):
                """w_{it+1} = z0 + n3^T w_it  (last iter emits mneg = -M)."""
                X = ctx[c]
                src = X["z0"] if it == 0 else X[f"w{it}"]
                wp_ = ps_nn.tile([128, 384], f32, tag="nn", name="wp")
                for i in range(6):
                    sl = slice(64 * i, 64 * i + 64)
                    nc.tensor.matmul(wp_[:, sl],
                                     lhsT=X["n3"][i // 3][:, 128 * (i % 3):128 * (i % 3) + 128],
                                     rhs=src[:, sl])
                if it < NIT - 1:
                    t_ = solvep.tile([128, 384], bf16, tag=f"w{it + 1}", name=f"w{it + 1}")
                    nc.vector.tensor_add(t_, wp_, X["z0"])
                    X[f"w{it + 1}"] = t_
                else:
                    t_ = solvep.tile([128, 384], bf16, tag="mneg", name="mneg")
                    nc.vector.scalar_tensor_tensor(
                        out=t_, in0=wp_, scalar=-1.0, in1=X["z0"],
                        op0=Alu.mult, op1=Alu.subtract)
                    X["mneg"] = t_

            def stage_ost(c):
                """O^T -> outT tile ; state update -> st_list[c+1]."""
                X = ctx[c]
                ops = ps_nn.tile([128, 384], f32, tag="nn", name="ops")
                for i in range(6):
                    p, sub = divmod(i, 2)
                    po = 64 * sub
                    sl = slice(po, po + 64)
                    osl = slice(128 * p, 128 * p + 128)
                    nc.tensor.matmul(ops[sl, osl], lhsT=X["knp"][p][:, sl],
                                     rhs=X["gsb"][i][:, 384:512],
                                     start=True, stop=False)
                    nc.tensor.matmul(ops[sl, osl],
                                     lhsT=X["mneg"][:, 64 * i:64 * i + 64],
                                     rhs=X["gsb"][i][:, 256:384],
                                     start=False, stop=False)
                    nc.tensor.matmul(ops[sl, osl],
                                     lhsT=st_list[c][sl, 64 * p:64 * p + 64],
                                     rhs=X["tsb"][p][po:po + 64, 128:256],
                                     start=False, stop=True)
                ot = outp.tile([128, 384], bf16, tag="outT", name="outT")
                nc.scalar.copy(ot, ops)
                X["outT"] = ot

                sp = ps_nn.tile([128, 192], f32, tag="nn", name="sps")
                for i in range(6):
                    p, sub = divmod(i, 2)
                    po = 64 * sub
                    psl = slice(po, po + 64)
                    fsl = slice(64 * p, 64 * p + 64)
                    nc.tensor.matmul(sp[psl, fsl], lhsT=X["vnp"][p][:, psl],
                                     rhs=X["knp"][p][:, psl],
                                     start=True, stop=False)
                    nc.tensor.matmul(sp[psl, fsl], lhsT=X["knp"][p][:, psl],
                                     rhs=X["mneg"][:, 64 * i:64 * i + 64],
                                     start=False, stop=True)
                stg = stp.tile([128, 192], bf16, tag="stg")
                nc.scalar.mul(stg, st_list[c], GN)
                st_new = stp.tile([128, 192], bf16, tag=f"st{(c + 1) % 2}", name=f"stn{c}")
                nc.vector.scalar_tensor_tensor(
                    out=st_new, in0=sp, scalar=GN, in1=stg,
                    op0=Alu.mult, op1=Alu.add)
                st_list.append(st_new)

            def stage_yout(c):
                """y[t0:t0+128] = outT^T @ wP (bf16 out)."""
                X = ctx[c]
                t0 = NC_ * c
                y_sb = yp.tile([128, 768], bf16, tag="ysb")
                for nb in range(2):
                    ypp = ps_nn.tile([128, 384], f32, tag="nn", name="ypp")
                    for k in range(3):
                        nc.tensor.matmul(
                            ypp[:, :],
                            lhsT=X["outT"][:, 128 * k:128 * k + 128],
                            rhs=wP_sb[:, k, 384 * nb:384 * nb + 384],
                            start=(k == 0), stop=(k == 2),
                        )
                    nc.scalar.copy(y_sb[:, 384 * nb:384 * nb + 384], ypp)
                nc.sync.dma_start(out=y[t0:t0 + 128, :], in_=y_sb)

            # ---- software-pipelined emission ----
            stage_qkv_mm(0)
            stage_qkv_norm(0)
            stage_tr(0)
            stage_gram(0)
            stage_nn(0)
            for c in range(NCH):
                stage_r(c)
                if c + 1 < NCH:
                    stage_qkv_mm(c + 1)
                stage_z0(c)
                if c + 1 < NCH:
                    stage_qkv_norm(c + 1)
                stage_w(c, 0)
                if c + 1 < NCH:
                    stage_tr(c + 1)
                stage_w(c, 1)
                if c + 1 < NCH:
                    stage_gram(c + 1)
                stage_ost(c)
                if c + 1 < NCH:
                    stage_nn(c + 1)
                stage_yout(c)

    nc.finalize()
    return nc


def _host_inputs(x, w_attn, w_proj):
    """Build the 8 per-core input maps (all heavy tensors bf16)."""
    import ml_dtypes
    bf = ml_dtypes.bfloat16
    in_maps = []
    gvec = np.zeros((128, 2), np.float32)
    p = np.arange(1, 129, dtype=np.float64)
    gvec[:, 0] = GAMMA ** p
    gvec[:, 1] = GAMMA ** (-p)
    ident = np.eye(128, dtype=np.float32).astype(bf)
    on = np.full((128, 128), -BP, np.float32)
    iu = np.triu(np.ones((128, 128), np.float32), 1)
    iui = np.triu(np.ones((128, 128), np.float32), 0)
    cmask = np.concatenate([on, iu, iui, iui], axis=1)
    for core in range(8):
        b, hg = divmod(core, 2)
        h0 = hg * HPC
        cols = []
        for blk in range(3):   # q, k, v column blocks of w_attn
            cols.append(w_attn[:, blk * C + h0 * HS: blk * C + (h0 + HPC) * HS])
        wA_s = np.concatenate(cols, axis=1).astype(bf)            # [768, 1152]
        wA_s = np.ascontiguousarray(
            wA_s.reshape(6, 128, 1152).transpose(1, 0, 2).reshape(128, 6 * 1152))
        wP_s = w_proj[h0 * HS:(h0 + HPC) * HS].astype(bf)         # [384, 768]
        wP_s = np.ascontiguousarray(
            wP_s.reshape(3, 128, 768).transpose(1, 0, 2).reshape(128, 3 * 768))
        xTb = x[b].T.astype(bf)                                   # [768, 1024]
        xTb = np.ascontiguousarray(
            xTb.reshape(6, 128, 8, 128).transpose(1, 2, 0, 3).reshape(128, 8 * 768))
        in_maps.append({
            "xT": xTb,
            "wA": wA_s,
            "wP": wP_s,
            "gvec": gvec,
            "cid": ident,
            "cmask": cmask,
        })
    return in_maps


def kernel(x, w_attn, w_proj):
    from concourse.bass_utils import run_bass_kernel_spmd

    if "nc" not in _cache:
        _cache["nc"] = _build_program()
    nc = _cache["nc"]

    x = np.asarray(x)
    in_maps = _host_inputs(x, np.asarray(w_attn), np.asarray(w_proj))
    res = run_bass_kernel_spmd(nc, in_maps, core_ids=list(range(8)))
    out = np.empty((B, T, C), np.float32)
    for b in range(B):
        out[b] = (x[b]
                  + res.results[2 * b]["y"].astype(np.float32)
                  + res.results[2 * b + 1]["y"].astype(np.float32))
    return out
